# revision 29
# baseline (speedup 1.0000x reference)
"""Causal self-attention (GPT-style block) on 8 Trainium2 NeuronCores.

Sharding: tensor-parallel over heads. 16 heads / 8 cores = 2 heads per core.
- c_attn column-parallel: each core computes q/k/v for its 2 heads (128
  channels each of q, k, v) from the full input x.
- attention: fully local per core (its 2 heads, all 4 batches).
- c_proj row-parallel: each core multiplies its 128 local channels by its
  128-row slice of w_proj, producing a full-shape partial output. The host
  sums the 8 partials and adds b_proj.

Device kernel notes (all matmuls contract over the partition dim):
- Matmul inputs use float32r (single-pass fp32 on the PE, 4x the fp32 rate;
  ~1.5e-4 input rounding, fp32 accumulate).
- x is fed pre-transposed + tiled from the host: xp[tb, p, kt, s] =
  x[(tb*512+s) token, (kt*128+p) channel] so stage 1 needs no transposes.
- q,k,v are produced channel-major ([chan, token]); v is then PE-transposed
  to token-major tiles with a ones column appended (vaug[.., 64]==1), so a
  single M=65 matmul accumulates both O^T = V^T E and the softmax
  denominator (row 64) per key tile.
- Scores are computed transposed: S^T[key, query] = (k^T).T @ q^T with the
  2 heads packed into the two 64-row halves of the PE array (row tiling).
- Softmax without max-subtraction (logits bounded ~|3| here): E =
  exp(S^T/8) on ACT, causal mask applied multiplicatively on the 4 partial
  (diagonal) key-tiles per query block.
- Normalization: r = 1/l on DVE, broadcast across the 64 head rows with a
  K=1 ones matmul on PE, multiply on DVE. Result lands channel-major in
  yT, which is exactly the stationary layout c_proj needs.
- c_proj is token-parallel: per batch, an on-device AllToAll exchanges Y^T
  slices (each core sends peer j its 2 head-channels for peer j's 256
  tokens), after which every core holds all 1024 channels for its own 256
  tokens and computes fully-reduced output rows with the full w_proj. This
  cuts per-core PSUM->SBUF eviction and output DMA 8x vs row-parallel
  partial sums.
"""

import numpy as np

P = 128
B = 4
T = 2048
BT = B * T            # 8192 tokens
C = 1024
KT = C // P           # 8 contraction tiles of 128 input channels
NTB = BT // 512       # 16 token blocks of 512
HD = 64               # head dim
NQ = T // 512         # 4 query blocks per batch
NCORES = 8

_CACHED = {}


def _build_nc():
    import concourse.mybir as mybir
    import concourse.tile as tile
    from concourse import bacc
    from concourse.masks import make_identity

    f32 = mybir.dt.float32
    f32r = mybir.dt.float32r
    EXP = mybir.ActivationFunctionType.Exp

    nc = bacc.Bacc("TRN2", target_bir_lowering=False, debug=False,
                   num_devices=NCORES)

    TPC = T // NCORES   # 256 tokens per core per batch (proj sharding)

    xp = nc.dram_tensor("xp", [NTB, P, KT, 512], f32r, kind="ExternalInput")
    wq = nc.dram_tensor("wq", [P, KT, P], f32r, kind="ExternalInput")
    wk = nc.dram_tensor("wk", [P, KT, P], f32r, kind="ExternalInput")
    wv = nc.dram_tensor("wv", [P, KT, P], f32r, kind="ExternalInput")
    wp = nc.dram_tensor("wp", [P, KT, C], f32r, kind="ExternalInput")
    bq = nc.dram_tensor("bq", [P, 1], f32, kind="ExternalInput")
    bk = nc.dram_tensor("bk", [P, 1], f32, kind="ExternalInput")
    bv = nc.dram_tensor("bv", [P, 1], f32, kind="ExternalInput")
    yp = nc.dram_tensor("yp", [B, 2, T // 2 // NCORES, C], f32, kind="ExternalOutput")

    with tile.TileContext(nc) as tc:
        with (
            tc.tile_pool(name="const", bufs=1) as const,
            tc.tile_pool(name="xt", bufs=2) as xt_pool,
            tc.tile_pool(name="slab", bufs=2) as slab_pool,
            tc.tile_pool(name="e", bufs=5) as e_pool,
            tc.tile_pool(name="nrm", bufs=4) as nrm_pool,
            tc.tile_pool(name="ob", bufs=3) as ob_pool,
            tc.tile_pool(name="yg", bufs=2) as yg_pool,
            tc.tile_pool(name="dram", bufs=1, space="DRAM") as dram_pool,
            tc.tile_pool(name="ps1", bufs=1, space="PSUM") as ps1_pool,
            tc.tile_pool(name="pss", bufs=2, space="PSUM") as pss_pool,
            tc.tile_pool(name="pso", bufs=2, space="PSUM") as pso_pool,
            tc.tile_pool(name="ppb", bufs=1, space="PSUM") as ppb_pool,
        ):
            TPH = TPC // 2   # 128 tokens per core per half-batch exchange
            g_in = [dram_pool.tile([NCORES, P, TPH], f32r, name=f"g_in{k}",
                                   tag=f"g_in{k}") for k in range(2 * B)]
            g_out = [dram_pool.tile([NCORES, P, TPH], f32r, name=f"g_out{k}",
                                    tag=f"g_out{k}") for k in range(2 * B)]

            # --- constants / weights resident in SBUF ---
            wq_sb = const.tile([P, KT, P], f32r)
            wk_sb = const.tile([P, KT, P], f32r)
            wv_sb = const.tile([P, KT, P], f32r)
            wp_sb = const.tile([P, KT, C], f32r)
            bq_sb = const.tile([P, 1], f32)
            bk_sb = const.tile([P, 1], f32)
            bv_sb = const.tile([P, 1], f32)
            nc.sync.dma_start(wq_sb[:], wq[:])
            nc.sync.dma_start(wk_sb[:], wk[:])
            nc.sync.dma_start(wv_sb[:], wv[:])
            nc.sync.dma_start(wp_sb[:], wp[:])
            nc.sync.dma_start(bq_sb[:], bq[:])
            nc.sync.dma_start(bk_sb[:], bk[:])
            nc.sync.dma_start(bv_sb[:], bv[:])

            ones_row_f = const.tile([1, HD], f32)
            nc.vector.memset(ones_row_f[:], 1.0)
            ones_row = const.tile([1, HD], f32r)
            nc.vector.tensor_copy(ones_row[:], ones_row_f[:])
            ones_v = const.tile([P, T // P, 1], f32)
            nc.vector.memset(ones_v[:], 1.0)
            ident = const.tile([P, P], f32)
            make_identity(nc, ident[:])

            # mask[p, s] = 1.0 if s >= p else 0.0 (keep upper-right triangle)
            # (built in f32 — gpsimd can't write f32r — then rounded over)
            mask_f = const.tile([P, P], f32)
            nc.gpsimd.memset(mask_f[:], 1.0)
            nc.gpsimd.affine_select(
                out=mask_f[:],
                in_=mask_f[:],
                compare_op=mybir.AluOpType.is_ge,
                fill=0.0,
                base=0,
                pattern=[[1, P]],
                channel_multiplier=-1,
            )
            mask_sb = const.tile([P, P], f32r)
            nc.vector.tensor_copy(mask_sb[:], mask_f[:])

            def emit_proj(k):
                # yg[p, cc, t]: channel cc*128+p of my token t of unit k
                yg = yg_pool.tile([P, NCORES, TPH], f32r, tag="yg")
                nc.sync.dma_start(yg[:], g_out[k].rearrange("c p t -> p c t"))
                pp0 = ppb_pool.tile([P, 512], f32, tag="ppb")
                pp1 = ppb_pool.tile([P, 512], f32, tag="ppb")
                for ct in range(KT):
                    nc.tensor.matmul(pp0[:], yg[:, ct, :], wp_sb[:, ct, 0:512],
                                     start=(ct == 0), stop=(ct == KT - 1))
                for ct in range(KT):
                    nc.tensor.matmul(pp1[:], yg[:, ct, :], wp_sb[:, ct, 512:C],
                                     start=(ct == 0), stop=(ct == KT - 1))
                ob = ob_pool.tile([P, C], f32, tag="ob")
                nc.vector.tensor_copy(ob[:, 0:512], pp0[:])
                nc.vector.tensor_copy(ob[:, 512:C], pp1[:])
                nc.sync.dma_start(yp[k // 2, k % 2, :, :], ob[:])

            def emit_exchange(k, yTh):
                # peer j gets my 2 head-channels for its 128 tokens of unit k
                for j in range(NCORES):
                    nc.sync.dma_start(g_in[k][j], yTh[:, j * TPH:(j + 1) * TPH])
                nc.gpsimd.collective_compute(
                    "AllToAll",
                    mybir.AluOpType.bypass,
                    replica_groups=[list(range(NCORES))],
                    ins=[g_in[k][:]],
                    outs=[g_out[k][:]],
                )

            for b in range(B):
                # --- stage 1: q^T, k^T, v^T (channel-major, f32r) ---
                qT = slab_pool.tile([P, T], f32r, tag="qT")
                kT = slab_pool.tile([P, T], f32r, tag="kT")
                vT = slab_pool.tile([P, T], f32, tag="scratch")
                # token-major v with ones cols at 64 (h0) and 129 (h1)
                vaug = slab_pool.tile([P, T // P, 2 * HD + 2], f32r, tag="vaug")
                nc.vector.tensor_copy(vaug[:, :, HD:HD + 1], ones_v[:])
                nc.vector.tensor_copy(vaug[:, :, 2 * HD + 1:2 * HD + 2], ones_v[:])

                for lb in range(NQ):
                    tb = b * NQ + lb
                    xt = xt_pool.tile([P, KT, 512], f32r)
                    nc.sync.dma_start(xt[:], xp[tb])
                    sl = slice(lb * 512, (lb + 1) * 512)

                    for w_sb, b_sb, dst in ((wq_sb, bq_sb, qT),
                                            (wk_sb, bk_sb, kT),
                                            (wv_sb, bv_sb, vT)):
                        ps = ps1_pool.tile([P, 512], f32, tag="ps1")
                        for kt in range(KT):
                            nc.tensor.matmul(ps[:], w_sb[:, kt, :], xt[:, kt, :],
                                             start=(kt == 0), stop=(kt == KT - 1))
                        nc.vector.tensor_scalar_add(dst[:, sl], ps[:], b_sb[:])

                    # transpose v to token-major [tok, chan] tiles
                    for t4 in range(4):
                        j = lb * 4 + t4
                        pst = ps1_pool.tile([P, P], f32, tag="ps1")
                        nc.tensor.transpose(pst[:], vT[:, j * P:(j + 1) * P], ident[:])
                        nc.vector.tensor_copy(vaug[:, j, 0:HD], pst[:, 0:HD])
                        nc.vector.tensor_copy(vaug[:, j, HD + 1:2 * HD + 1],
                                              pst[:, HD:P])

                # --- stage 2: attention, per query block ---
                for i in range(NQ):
                    if i % 2 == 0:
                        yT = slab_pool.tile([P, T // 2], f32r, tag="scratch",
                                            name=f"yT_{b}_{i // 2}")
                    isl = slice((i % 2) * 512, (i % 2 + 1) * 512)
                    nj = 4 * (i + 1)
                    po0 = pso_pool.tile([P, 512], f32, tag="pso")
                    po1 = pso_pool.tile([P, 512], f32, tag="pso")
                    for j in range(nj):
                        jsl = slice(j * P, (j + 1) * P)
                        jj = j - 4 * i
                        # diagonal tiles: queries below q0 can't see this key
                        # tile — compute only the [q0, 512) query range
                        q0 = max(0, jj) * P
                        qsl = slice(i * 512 + q0, (i + 1) * 512)
                        vsl = slice(q0, 512)
                        # both heads' scores side by side in one 2-bank
                        # psum tile -> a single exp per key tile
                        psp = pss_pool.tile([P, 1024], f32, tag="pss")
                        nc.tensor.matmul(psp[:, vsl], kT[0:HD, jsl], qT[0:HD, qsl],
                                         start=True, stop=True, tile_position=(0, 0))
                        v1 = slice(512 + q0, 1024)
                        nc.tensor.matmul(psp[:, v1], kT[HD:P, jsl], qT[HD:P, qsl],
                                         start=True, stop=True, tile_position=(HD, 0))
                        ep = e_pool.tile([P, 1024], f32r, tag="e")
                        if q0 == 0:
                            nc.scalar.activation(ep[:], psp[:], EXP, scale=0.125)
                        else:
                            nc.scalar.activation(ep[:, vsl], psp[:, vsl], EXP,
                                                 scale=0.125)
                            nc.scalar.activation(ep[:, v1], psp[:, v1], EXP,
                                                 scale=0.125)
                        if jj >= 0:
                            for q in (q0, 512 + q0):
                                msl = slice(q, q + P)
                                nc.vector.tensor_mul(ep[:, msl], ep[:, msl],
                                                     mask_sb[:])
                        st, sp = (j == 0), (j == nj - 1)
                        nc.tensor.matmul(po0[0:HD + 1, vsl], vaug[:, j, 0:HD + 1],
                                         ep[:, vsl], start=st, stop=sp)
                        nc.tensor.matmul(po1[0:HD + 1, vsl],
                                         vaug[:, j, HD + 1:2 * HD + 2], ep[:, v1],
                                         start=st, stop=sp)

                    # normalize: yT[head, isl] = O^T * (1/l) broadcast over rows
                    with nc.allow_low_precision(reason="f32r matmul inputs"):
                        r0 = nrm_pool.tile([1, 512], f32r, tag="r")
                        r1 = nrm_pool.tile([1, 512], f32r, tag="r")
                        nc.vector.reciprocal(r0[:], po0[HD:HD + 1, :])
                        nc.vector.reciprocal(r1[:], po1[HD:HD + 1, :])
                    pb0 = ppb_pool.tile([P, 512], f32, tag="ppb")
                    pb1 = ppb_pool.tile([P, 512], f32, tag="ppb")
                    nc.tensor.matmul(pb0[0:HD, :], ones_row[:], r0[:],
                                     start=True, stop=True)
                    nc.tensor.matmul(pb1[0:HD, :], ones_row[:], r1[:],
                                     start=True, stop=True)
                    rb0 = nrm_pool.tile([HD, 512], f32, tag="rb")
                    rb1 = nrm_pool.tile([HD, 512], f32, tag="rb")
                    nc.vector.tensor_copy(rb0[:], pb0[0:HD, :])
                    nc.vector.tensor_copy(rb1[:], pb1[0:HD, :])
                    nc.vector.tensor_mul(yT[0:HD, isl], po0[0:HD, :], rb0[:])
                    nc.vector.tensor_mul(yT[HD:P, isl], po1[0:HD, :], rb1[:])

                    if i % 2 == 1:
                        k = 2 * b + i // 2
                        emit_exchange(k, yT)
                        if k >= 3:
                            emit_proj(k - 3)
            for k in range(2 * B - 3, 2 * B):
                emit_proj(k)

    nc.compile()
    return nc


def _prep_inputs(x, w_attn, b_attn, w_proj):
    x = np.asarray(x, dtype=np.float32)
    w_attn = np.asarray(w_attn, dtype=np.float32)
    b_attn = np.asarray(b_attn, dtype=np.float32)
    w_proj = np.asarray(w_proj, dtype=np.float32)

    x_flat = x.reshape(BT, C)
    # xp[tb, p, kt, s] = x_flat[tb*512+s, kt*128+p]
    xp = np.ascontiguousarray(
        x_flat.T.reshape(KT, P, NTB, 512).transpose(2, 1, 0, 3))

    in_maps = []
    for c in range(NCORES):
        cols = slice(P * c, P * (c + 1))

        def wslice(off):
            w = w_attn[:, off + P * c: off + P * (c + 1)]   # [1024, 128]
            return np.ascontiguousarray(w.reshape(KT, P, P).transpose(1, 0, 2))

        in_maps.append({
            "xp": xp,
            "wq": wslice(0),
            "wk": wslice(C),
            "wv": wslice(2 * C),
            "wp": np.ascontiguousarray(w_proj.reshape(KT, P, C).transpose(1, 0, 2)),
            "bq": np.ascontiguousarray(b_attn[cols]).reshape(P, 1),
            "bk": np.ascontiguousarray(b_attn[C + P * c: C + P * (c + 1)]).reshape(P, 1),
            "bv": np.ascontiguousarray(b_attn[2 * C + P * c: 2 * C + P * (c + 1)]).reshape(P, 1),
        })
    return in_maps


def kernel(x, w_attn, b_attn, w_proj, b_proj):
    from concourse.bass_utils import run_bass_kernel_spmd

    if "nc" not in _CACHED:
        _CACHED["nc"] = _build_nc()
    nc = _CACHED["nc"]

    in_maps = _prep_inputs(x, w_attn, b_attn, w_proj)
    res = run_bass_kernel_spmd(nc, in_maps, core_ids=list(range(NCORES)))

    # core c holds tokens [h*1024 + c*128, +128) of each batch half h
    y = np.empty((B, T, C), dtype=np.float32)
    for c in range(NCORES):
        part = res.results[c]["yp"]          # [B, 2, 128, C]
        for h in range(2):
            y[:, h * (T // 2) + c * 128: h * (T // 2) + (c + 1) * 128, :] = part[:, h]
    y += np.asarray(b_proj, dtype=np.float32)
    return y


# revision 33
# speedup vs baseline: 1.0072x; 1.0072x over previous
"""Causal self-attention (GPT-style block) on 8 Trainium2 NeuronCores.

Sharding: tensor-parallel over heads. 16 heads / 8 cores = 2 heads per core.
- c_attn column-parallel: each core computes q/k/v for its 2 heads (128
  channels each of q, k, v) from the full input x.
- attention: fully local per core (its 2 heads, all 4 batches).
- c_proj row-parallel: each core multiplies its 128 local channels by its
  128-row slice of w_proj, producing a full-shape partial output. The host
  sums the 8 partials and adds b_proj.

Device kernel notes (all matmuls contract over the partition dim):
- Matmul inputs use float32r (single-pass fp32 on the PE, 4x the fp32 rate;
  ~1.5e-4 input rounding, fp32 accumulate).
- x is fed pre-transposed + tiled from the host: xp[tb, p, kt, s] =
  x[(tb*512+s) token, (kt*128+p) channel] so stage 1 needs no transposes.
- q,k,v are produced channel-major ([chan, token]); v is then PE-transposed
  to token-major tiles with a ones column appended (vaug[.., 64]==1), so a
  single M=65 matmul accumulates both O^T = V^T E and the softmax
  denominator (row 64) per key tile.
- Scores are computed transposed: S^T[key, query] = (k^T).T @ q^T with the
  2 heads packed into the two 64-row halves of the PE array (row tiling).
- Softmax without max-subtraction (logits bounded ~|3| here): E =
  exp(S^T/8) on ACT, causal mask applied multiplicatively on the 4 partial
  (diagonal) key-tiles per query block.
- Normalization: r = 1/l on DVE, broadcast across the 64 head rows with a
  K=1 ones matmul on PE, multiply on DVE. Result lands channel-major in
  yT, which is exactly the stationary layout c_proj needs.
- c_proj is token-parallel: per batch, an on-device AllToAll exchanges Y^T
  slices (each core sends peer j its 2 head-channels for peer j's 256
  tokens), after which every core holds all 1024 channels for its own 256
  tokens and computes fully-reduced output rows with the full w_proj. This
  cuts per-core PSUM->SBUF eviction and output DMA 8x vs row-parallel
  partial sums.
"""

import numpy as np

P = 128
B = 4
T = 2048
BT = B * T            # 8192 tokens
C = 1024
KT = C // P           # 8 contraction tiles of 128 input channels
NTB = BT // 512       # 16 token blocks of 512
HD = 64               # head dim
NQ = T // 512         # 4 query blocks per batch
NCORES = 8

_CACHED = {}


def _build_nc():
    import concourse.mybir as mybir
    import concourse.tile as tile
    from concourse import bacc
    from concourse.masks import make_identity

    f32 = mybir.dt.float32
    f32r = mybir.dt.float32r
    EXP = mybir.ActivationFunctionType.Exp

    nc = bacc.Bacc("TRN2", target_bir_lowering=False, debug=False,
                   num_devices=NCORES)

    TPC = T // NCORES   # 256 tokens per core per batch (proj sharding)

    xp = nc.dram_tensor("xp", [NTB, P, KT, 512], f32r, kind="ExternalInput")
    wq = nc.dram_tensor("wq", [P, KT, P], f32r, kind="ExternalInput")
    wk = nc.dram_tensor("wk", [P, KT, P], f32r, kind="ExternalInput")
    wv = nc.dram_tensor("wv", [P, KT, P], f32r, kind="ExternalInput")
    wp = nc.dram_tensor("wp", [P, KT, C], f32r, kind="ExternalInput")
    bq = nc.dram_tensor("bq", [P, 1], f32, kind="ExternalInput")
    bk = nc.dram_tensor("bk", [P, 1], f32, kind="ExternalInput")
    bv = nc.dram_tensor("bv", [P, 1], f32, kind="ExternalInput")
    yp = nc.dram_tensor("yp", [B, 2, T // 2 // NCORES, C], f32, kind="ExternalOutput")

    with tile.TileContext(nc) as tc:
        with (
            tc.tile_pool(name="const", bufs=1) as const,
            tc.tile_pool(name="xt", bufs=2) as xt_pool,
            tc.tile_pool(name="slab", bufs=2) as slab_pool,
            tc.tile_pool(name="e", bufs=5) as e_pool,
            tc.tile_pool(name="nrm", bufs=2) as nrm_pool,
            tc.tile_pool(name="ob", bufs=3) as ob_pool,
            tc.tile_pool(name="yg", bufs=3) as yg_pool,
            tc.tile_pool(name="dram", bufs=1, space="DRAM") as dram_pool,
            tc.tile_pool(name="ps1", bufs=1, space="PSUM") as ps1_pool,
            tc.tile_pool(name="pss", bufs=2, space="PSUM") as pss_pool,
            tc.tile_pool(name="pso", bufs=2, space="PSUM") as pso_pool,
            tc.tile_pool(name="ppb", bufs=1, space="PSUM") as ppb_pool,
        ):
            TPH = TPC // 2   # 128 tokens per core per half-batch exchange
            g_in = [dram_pool.tile([NCORES, P, TPH], f32r, name=f"g_in{k}",
                                   tag=f"g_in{k}") for k in range(2 * B)]
            g_out = [dram_pool.tile([NCORES, P, TPH], f32r, name=f"g_out{k}",
                                    tag=f"g_out{k}") for k in range(2 * B)]

            # --- constants / weights resident in SBUF ---
            wq_sb = const.tile([P, KT, P], f32r)
            wk_sb = const.tile([P, KT, P], f32r)
            wv_sb = const.tile([P, KT, P], f32r)
            wp_sb = const.tile([P, KT, C], f32r)
            bq_sb = const.tile([P, 1], f32)
            bk_sb = const.tile([P, 1], f32)
            bv_sb = const.tile([P, 1], f32)
            nc.sync.dma_start(wq_sb[:], wq[:])
            nc.sync.dma_start(wk_sb[:], wk[:])
            nc.sync.dma_start(wv_sb[:], wv[:])
            nc.sync.dma_start(bq_sb[:], bq[:])
            nc.sync.dma_start(bk_sb[:], bk[:])
            nc.sync.dma_start(bv_sb[:], bv[:])

            ones_row_f = const.tile([1, HD], f32)
            nc.vector.memset(ones_row_f[:], 1.0)
            ones_row = const.tile([1, HD], f32r)
            nc.vector.tensor_copy(ones_row[:], ones_row_f[:])
            ones_v = const.tile([P, T // P, 1], f32)
            nc.vector.memset(ones_v[:], 1.0)
            ident = const.tile([P, P], f32)
            make_identity(nc, ident[:])

            # mask[p, s] = 1.0 if s >= p else 0.0 (keep upper-right triangle)
            # (built in f32 — gpsimd can't write f32r — then rounded over)
            mask_f = const.tile([P, P], f32)
            nc.gpsimd.memset(mask_f[:], 1.0)
            nc.gpsimd.affine_select(
                out=mask_f[:],
                in_=mask_f[:],
                compare_op=mybir.AluOpType.is_ge,
                fill=0.0,
                base=0,
                pattern=[[1, P]],
                channel_multiplier=-1,
            )
            mask_sb = const.tile([P, P], f32r)
            nc.vector.tensor_copy(mask_sb[:], mask_f[:])

            wp_loaded = []

            def emit_proj(k):
                if not wp_loaded:
                    # deferred so the 4MiB w_proj load doesn't delay the
                    # startup xp streaming
                    nc.sync.dma_start(wp_sb[:], wp[:])
                    wp_loaded.append(True)
                # yg[p, cc, t]: channel cc*128+p of my token t of unit k
                yg = yg_pool.tile([P, NCORES, TPH], f32r, tag="yg")
                nc.sync.dma_start(yg[:], g_out[k].rearrange("c p t -> p c t"))
                pp0 = ppb_pool.tile([P, 512], f32, tag="ppb")
                pp1 = ppb_pool.tile([P, 512], f32, tag="ppb")
                for ct in range(KT):
                    nc.tensor.matmul(pp0[:], yg[:, ct, :], wp_sb[:, ct, 0:512],
                                     start=(ct == 0), stop=(ct == KT - 1))
                for ct in range(KT):
                    nc.tensor.matmul(pp1[:], yg[:, ct, :], wp_sb[:, ct, 512:C],
                                     start=(ct == 0), stop=(ct == KT - 1))
                ob = ob_pool.tile([P, C], f32, tag="ob")
                nc.vector.tensor_copy(ob[:, 0:512], pp0[:])
                nc.vector.tensor_copy(ob[:, 512:C], pp1[:])
                nc.sync.dma_start(yp[k // 2, k % 2, :, :], ob[:])

            def emit_exchange(k, yTh):
                # peer j gets my 2 head-channels for its 128 tokens of unit k
                for j in range(NCORES):
                    nc.sync.dma_start(g_in[k][j], yTh[:, j * TPH:(j + 1) * TPH])
                nc.gpsimd.collective_compute(
                    "AllToAll",
                    mybir.AluOpType.bypass,
                    replica_groups=[list(range(NCORES))],
                    ins=[g_in[k][:]],
                    outs=[g_out[k][:]],
                )

            for b in range(B):
                # --- stage 1: q^T, k^T, v^T (channel-major, f32r) ---
                qT = slab_pool.tile([P, T], f32r, tag="qT")
                kT = slab_pool.tile([P, T], f32r, tag="kT")
                vT = slab_pool.tile([P, T], f32, tag="scratch")
                # token-major v with ones cols at 64 (h0) and 129 (h1)
                vaug = slab_pool.tile([P, T // P, 2 * HD + 2], f32r, tag="vaug")
                nc.vector.tensor_copy(vaug[:, :, HD:HD + 1], ones_v[:])
                nc.vector.tensor_copy(vaug[:, :, 2 * HD + 1:2 * HD + 2], ones_v[:])

                for lb in range(NQ):
                    tb = b * NQ + lb
                    xt = xt_pool.tile([P, KT, 512], f32r)
                    nc.sync.dma_start(xt[:], xp[tb])
                    sl = slice(lb * 512, (lb + 1) * 512)

                    for w_sb, b_sb, dst in ((wq_sb, bq_sb, qT),
                                            (wk_sb, bk_sb, kT),
                                            (wv_sb, bv_sb, vT)):
                        ps = ps1_pool.tile([P, 512], f32, tag="ps1")
                        for kt in range(KT):
                            nc.tensor.matmul(ps[:], w_sb[:, kt, :], xt[:, kt, :],
                                             start=(kt == 0), stop=(kt == KT - 1))
                        nc.vector.tensor_scalar_add(dst[:, sl], ps[:], b_sb[:])

                    # transpose v to token-major [tok, chan] tiles
                    for t4 in range(4):
                        j = lb * 4 + t4
                        pst = ps1_pool.tile([P, P], f32, tag="ps1")
                        nc.tensor.transpose(pst[:], vT[:, j * P:(j + 1) * P], ident[:])
                        nc.vector.tensor_copy(vaug[:, j, 0:HD], pst[:, 0:HD])
                        nc.vector.tensor_copy(vaug[:, j, HD + 1:2 * HD + 1],
                                              pst[:, HD:P])

                # --- stage 2: attention, per query block ---
                for i in range(NQ):
                    if i % 2 == 0:
                        yT = slab_pool.tile([P, T // 2], f32r, tag="scratch",
                                            name=f"yT_{b}_{i // 2}")
                    isl = slice((i % 2) * 512, (i % 2 + 1) * 512)
                    nj = 4 * (i + 1)
                    po0 = pso_pool.tile([P, 512], f32, tag="pso")
                    po1 = pso_pool.tile([P, 512], f32, tag="pso")
                    for j in range(nj):
                        jsl = slice(j * P, (j + 1) * P)
                        jj = j - 4 * i
                        # diagonal tiles: queries below q0 can't see this key
                        # tile — compute only the [q0, 512) query range
                        q0 = max(0, jj) * P
                        qsl = slice(i * 512 + q0, (i + 1) * 512)
                        vsl = slice(q0, 512)
                        # both heads' scores side by side in one 2-bank
                        # psum tile -> a single exp per key tile
                        psp = pss_pool.tile([P, 1024], f32, tag="pss")
                        nc.tensor.matmul(psp[:, vsl], kT[0:HD, jsl], qT[0:HD, qsl],
                                         start=True, stop=True, tile_position=(0, 0))
                        v1 = slice(512 + q0, 1024)
                        nc.tensor.matmul(psp[:, v1], kT[HD:P, jsl], qT[HD:P, qsl],
                                         start=True, stop=True, tile_position=(HD, 0))
                        ep = e_pool.tile([P, 1024], f32r, tag="e")
                        if q0 == 0:
                            nc.scalar.activation(ep[:], psp[:], EXP, scale=0.125)
                        else:
                            nc.scalar.activation(ep[:, vsl], psp[:, vsl], EXP,
                                                 scale=0.125)
                            nc.scalar.activation(ep[:, v1], psp[:, v1], EXP,
                                                 scale=0.125)
                        if jj >= 0:
                            for q in (q0, 512 + q0):
                                msl = slice(q, q + P)
                                nc.vector.tensor_mul(ep[:, msl], ep[:, msl],
                                                     mask_sb[:])
                        st, sp = (j == 0), (j == nj - 1)
                        nc.tensor.matmul(po0[0:HD + 1, vsl], vaug[:, j, 0:HD + 1],
                                         ep[:, vsl], start=st, stop=sp)
                        nc.tensor.matmul(po1[0:HD + 1, vsl],
                                         vaug[:, j, HD + 1:2 * HD + 2], ep[:, v1],
                                         start=st, stop=sp)

                    # normalize: yT[head, isl] = O^T * (1/l) broadcast over rows
                    with nc.allow_low_precision(reason="f32r matmul inputs"):
                        r0 = nrm_pool.tile([1, 512], f32r, tag="r")
                        r1 = nrm_pool.tile([1, 512], f32r, tag="r")
                        nc.vector.reciprocal(r0[:], po0[HD:HD + 1, :])
                        nc.vector.reciprocal(r1[:], po1[HD:HD + 1, :])
                    pb0 = ppb_pool.tile([P, 512], f32, tag="ppb")
                    pb1 = ppb_pool.tile([P, 512], f32, tag="ppb")
                    nc.tensor.matmul(pb0[0:HD, :], ones_row[:], r0[:],
                                     start=True, stop=True)
                    nc.tensor.matmul(pb1[0:HD, :], ones_row[:], r1[:],
                                     start=True, stop=True)
                    rb0 = nrm_pool.tile([HD, 512], f32, tag="rb")
                    rb1 = nrm_pool.tile([HD, 512], f32, tag="rb")
                    nc.vector.tensor_copy(rb0[:], pb0[0:HD, :])
                    nc.vector.tensor_copy(rb1[:], pb1[0:HD, :])
                    nc.vector.tensor_mul(yT[0:HD, isl], po0[0:HD, :], rb0[:])
                    nc.vector.tensor_mul(yT[HD:P, isl], po1[0:HD, :], rb1[:])

                    if i % 2 == 1:
                        k = 2 * b + i // 2
                        emit_exchange(k, yT)
                        if k >= 3:
                            emit_proj(k - 3)
            for k in range(2 * B - 3, 2 * B):
                emit_proj(k)

    nc.compile()
    return nc


def _prep_inputs(x, w_attn, b_attn, w_proj):
    x = np.asarray(x, dtype=np.float32)
    w_attn = np.asarray(w_attn, dtype=np.float32)
    b_attn = np.asarray(b_attn, dtype=np.float32)
    w_proj = np.asarray(w_proj, dtype=np.float32)

    x_flat = x.reshape(BT, C)
    # xp[tb, p, kt, s] = x_flat[tb*512+s, kt*128+p]
    xp = np.ascontiguousarray(
        x_flat.T.reshape(KT, P, NTB, 512).transpose(2, 1, 0, 3))

    in_maps = []
    for c in range(NCORES):
        cols = slice(P * c, P * (c + 1))

        def wslice(off):
            w = w_attn[:, off + P * c: off + P * (c + 1)]   # [1024, 128]
            return np.ascontiguousarray(w.reshape(KT, P, P).transpose(1, 0, 2))

        in_maps.append({
            "xp": xp,
            "wq": wslice(0),
            "wk": wslice(C),
            "wv": wslice(2 * C),
            "wp": np.ascontiguousarray(w_proj.reshape(KT, P, C).transpose(1, 0, 2)),
            "bq": np.ascontiguousarray(b_attn[cols]).reshape(P, 1),
            "bk": np.ascontiguousarray(b_attn[C + P * c: C + P * (c + 1)]).reshape(P, 1),
            "bv": np.ascontiguousarray(b_attn[2 * C + P * c: 2 * C + P * (c + 1)]).reshape(P, 1),
        })
    return in_maps


def kernel(x, w_attn, b_attn, w_proj, b_proj):
    from concourse.bass_utils import run_bass_kernel_spmd

    if "nc" not in _CACHED:
        _CACHED["nc"] = _build_nc()
    nc = _CACHED["nc"]

    in_maps = _prep_inputs(x, w_attn, b_attn, w_proj)
    res = run_bass_kernel_spmd(nc, in_maps, core_ids=list(range(NCORES)))

    # core c holds tokens [h*1024 + c*128, +128) of each batch half h
    y = np.empty((B, T, C), dtype=np.float32)
    for c in range(NCORES):
        part = res.results[c]["yp"]          # [B, 2, 128, C]
        for h in range(2):
            y[:, h * (T // 2) + c * 128: h * (T // 2) + (c + 1) * 128, :] = part[:, h]
    y += np.asarray(b_proj, dtype=np.float32)
    return y
